# revision 2
# baseline (speedup 1.0000x reference)
"""Trainium2 Bass kernel for nn_BallModel: 10M-step ballistic trajectory.

The reference recurrence (pos += vel*dt; vel += g*dt, recording pos) has the
closed form
    pos_i = pos0 + i*dt*vel0 + g*dt^2 * i*(i-1)/2  =  A + B*i + C*i^2
with A = pos0, B = dt*vel0 - C, C = (g*dt)*dt/2 (per component; C_x = 0).

Output is [10_000_000, 2] f32 (~80 MB), interleaved x,y.  Each of the 8 cores
produces a contiguous 2.5M-element slice (10 MB) -> memory-bound at the
per-core HBM write bandwidth (~358 GB/s line rate => ~28 us drain floor).

v2 layout: PARTITION-CONTIGUOUS.  Core-local element index
    e = p*W + col,   W = 19532 = 38*512 + 76   (out DRAM declared [128, W])
so each partition owns one contiguous W*4 = 78,128-byte run of HBM, and the
whole 10 MB ships in 10 column-slice DMAs of ~1 MB whose descriptors are 8 KB
per partition (vs 2 KB chunk-interleaved in v1) -> near-zero descriptor /
issue overhead on the sync engine.  128*W = 2,500,096 covers the core's
2.5M elements with 96 elements of overlap into the next core (host trims).

Pair index i = e>>1 = Q0 + p*(W/2) + 256*b + jb where b = col>>9 indexes the
39 512-column blocks (block 38 is 76 cols) and jb = (col&511)>>1, comp =
col&1 alternate x/y down the columns exactly as in v1, so the SAME shared
rhs table [K=10, 512] drives every block; only the per-(block, partition)
stationary lhsT tables change (q = Q0 + p*9766 + 256*b).

Everything is generated by ONE K=10 bf16 matmul per block (PE streams N
columns/cycle regardless of K).  Values wider than bf16's 8 mantissa bits
are split into 2-3 bf16 rows (hi/lo/lo2) whose products accumulate exactly
in the fp32 PSUM accumulator, so the result is fp32-faithful (~1e-7 rel of
the f64 closed form).  Blocks are processed in 10 groups of 4 (last group 3
blocks / 1100 cols): 4 matmuls -> one PSUM[128,2048] tile (4 banks) -> one
copy to SBUF (groups alternate vector/scalar engines so each engine sees a
single-semaphore dependency chain) -> one 1 MB output DMA.  Every
instruction carries at most ONE cross-engine wait, minimizing the standalone
EVENT_SEMAPHORE instructions Bacc.generate_event_semaphores spills (in v1
those cost ~8.6 us of post-drain epilogue and ~3.6 us of preamble).

Structural notes kept from v1:
 - built on bacc.Bacc, NOT raw bass.Bass, so that legalization runs;
 - every group gets its own SBUF output tile so copies carry no WAR waits;
 - input tables ship as a small head DMA (rh + first 2 groups' lhsT) that
   gates the first matmul, with the lhsT tail loading concurrently behind;
 - all DMAs on the sync HWDGE queue (gpsimd SWDGE stalls; scalar HWDGE
   hard-hangs the device).
"""

import sys
import types

import ml_dtypes
import numpy as np

import concourse.bacc as bacc
import concourse.bass as bass
import concourse.mybir as mybir
from concourse.bass_utils import run_bass_kernel_spmd
from concourse.tile import TileContext

# ---- problem constants (hardcoded; kernel.py must be self-contained) ----
N_PAIRS = 10_000_000
ELEMS = 2 * N_PAIRS  # 20,000,000 interleaved f32 values
N_CORES = 8
CE = ELEMS // N_CORES  # 2,500,000 elements per core
P = 128  # partitions
COLS = 512  # one PSUM bank of f32
W = 19532  # elems per partition per core (even; 38*512 + 76)
NB = 39  # column blocks per core (38 full + one 76-col partial)
LAST_BLOCK_COLS = W - 38 * COLS  # 76
K = 10  # matmul contraction rows
GROUPS = [list(range(4 * g, 4 * g + 4)) for g in range(9)] + [[36, 37, 38]]
HEAD_BLOCKS = 8  # first two groups' lhsT ride the fast head DMA

# fp32-rounded constants, matching the reference's fp32 parameter rounding
DT = float(np.float32(0.01))
GDT_Y = float(np.float32(np.float32(-9.81) * np.float32(0.01)))  # fp32(g_y*dt)
C_Y = GDT_Y * DT / 2.0  # i^2 coefficient for y

_bf16 = ml_dtypes.bfloat16

# exposed for test.py introspection (exec_time_ns etc.)
LAST_RESULTS = None


def _ensure_axon_hooks_stub():
    """bass_utils imports antenv.axon_hooks when BASS_TRACE is set; some
    images lack that module.  Register a stub that degrades to the untraced
    path instead of crashing (test.py replaces it with a real NTFF hook)."""
    try:
        import antenv.axon_hooks  # noqa: F401

        return
    except ImportError:
        pass
    try:
        import antenv  # noqa: F401
    except ImportError:
        return
    stub = types.ModuleType("antenv.axon_hooks")
    stub.get_axon_ntff_profile_hook = lambda: None
    stub.set_axon_ntff_profile_hook = lambda h: None
    sys.modules["antenv.axon_hooks"] = stub


def _build_program() -> bass.Bass:
    # Bacc (not raw Bass): its finalize pipeline runs the sync-wait
    # legalization and register allocation walrus requires.
    nc = bacc.Bacc("TRN2", target_bir_lowering=False)
    hd = nc.declare_dram_parameter(
        "hd", [K, COLS + HEAD_BLOCKS * P], mybir.dt.bfloat16, isOutput=False
    )
    lt_t = nc.declare_dram_parameter(
        "lt_t", [K, (NB - HEAD_BLOCKS) * P], mybir.dt.bfloat16, isOutput=False
    )
    out = nc.declare_dram_parameter("out", [P, W], mybir.dt.float32, isOutput=True)

    with TileContext(nc) as tc:
        with (
            tc.tile_pool(name="const", bufs=1) as cpool,
            tc.tile_pool(name="work", bufs=1) as wpool,
            tc.tile_pool(name="psum_a", bufs=1, space="PSUM") as ppool_a,
            tc.tile_pool(name="psum_b", bufs=1, space="PSUM") as ppool_b,
        ):
            hd_s = cpool.tile([K, COLS + HEAD_BLOCKS * P], mybir.dt.bfloat16)
            ltt_s = cpool.tile([K, (NB - HEAD_BLOCKS) * P], mybir.dt.bfloat16)
            nc.sync.dma_start(hd_s[:], hd[:])
            nc.sync.dma_start(ltt_s[:], lt_t[:])
            rh_s = hd_s[:, :COLS]

            def lhsT(b):
                if b < HEAD_BLOCKS:
                    return hd_s[:, COLS + b * P : COLS + (b + 1) * P]
                b -= HEAD_BLOCKS
                return ltt_s[:, b * P : (b + 1) * P]

            for g, blocks in enumerate(GROUPS):
                ppool = ppool_b if g % 2 else ppool_a
                pt = ppool.tile([P, 4 * COLS], mybir.dt.float32, name="pt", tag="pt")
                ncols = 0
                for idx, b in enumerate(blocks):
                    bc = LAST_BLOCK_COLS if b == NB - 1 else COLS
                    nc.tensor.matmul(
                        pt[:, idx * COLS : idx * COLS + bc],
                        lhsT(b),
                        rh_s[:, :bc],
                        start=True,
                        stop=True,
                    )
                    ncols = idx * COLS + bc
                ot = wpool.tile([P, ncols], mybir.dt.float32, name=f"ot{g}", tag=f"ot{g}")
                if g % 2:
                    nc.scalar.copy(ot[:], pt[:, :ncols])
                else:
                    nc.vector.tensor_copy(ot[:], pt[:, :ncols])
                c0 = blocks[0] * COLS
                nc.sync.dma_start(out[:, c0 : c0 + ncols], ot[:])
    nc.finalize()  # runs Bacc.compile(): reg alloc + sync-wait legalization
    return nc


def _split_bf16(x: np.ndarray, n: int):
    """Split x into n bf16 parts summing (nearly) exactly to x."""
    parts = []
    rem = np.asarray(x, dtype=np.float64).copy()
    for _ in range(n):
        p = rem.astype(_bf16)
        parts.append(p)
        rem = rem - p.astype(np.float64)
    return parts


def _host_tables(pos0: np.ndarray, vel0: np.ndarray):
    """Build per-core input tables (float64 math, cast at the end)."""
    ax, ay = float(pos0[0]), float(pos0[1])
    bx_c = DT * float(vel0[0])  # B_x (C_x = 0)
    by_c = DT * float(vel0[1]) - C_Y  # B_y

    # fixed rhs column patterns (jb < 256 within every 512-col block)
    ce = np.arange(COLS)
    j = (ce >> 1).astype(np.float64)
    odd = (ce & 1).astype(np.float64)
    even = 1.0 - odd
    jodd = (j * odd).astype(_bf16)  # exact: j < 256
    resid = np.where(ce & 1 == 1, C_Y * j * j, bx_c * j)
    resid_hi, resid_lo = _split_bf16(resid, 2)
    rh_np = np.stack(
        [
            jodd,
            jodd,
            resid_hi,
            resid_lo,
            odd.astype(_bf16),
            odd.astype(_bf16),
            odd.astype(_bf16),
            even.astype(_bf16),
            even.astype(_bf16),
            even.astype(_bf16),
        ]
    )  # [K, COLS]

    in_maps = []
    b_idx = np.arange(NB, dtype=np.float64)[:, None]  # [NB, 1]
    p_idx = np.arange(P, dtype=np.float64)[None, :]  # [1, P]
    for k in range(N_CORES):
        q = k * (CE // 2) + p_idx * (W // 2) + b_idx * (COLS // 2)  # [NB, P]
        s1_hi, s1_lo = _split_bf16(by_c + 2.0 * C_Y * q, 2)
        ones = np.ones_like(s1_hi)
        by3 = _split_bf16(ay + by_c * q + C_Y * q * q, 3)
        bx3 = _split_bf16(ax + bx_c * q, 3)
        rows = [s1_hi, s1_lo, ones, ones] + by3 + bx3
        lt_np = np.stack([r.reshape(-1) for r in rows])  # [K, NB*P]
        in_maps.append(
            {
                "hd": np.ascontiguousarray(
                    np.concatenate([rh_np, lt_np[:, : HEAD_BLOCKS * P]], axis=1)
                ),
                "lt_t": np.ascontiguousarray(lt_np[:, HEAD_BLOCKS * P :]),
            }
        )
    return in_maps


def kernel(ball_mass, ball_initial_position, ball_initial_velocity) -> np.ndarray:
    global LAST_RESULTS
    pos0 = np.asarray(ball_initial_position, dtype=np.float32)
    vel0 = np.asarray(ball_initial_velocity, dtype=np.float32)

    _ensure_axon_hooks_stub()
    nc = _build_program()
    in_maps = _host_tables(pos0, vel0)
    res = run_bass_kernel_spmd(nc, in_maps, core_ids=list(range(N_CORES)))
    LAST_RESULTS = res

    parts = [
        np.asarray(r["out"], dtype=np.float32).reshape(-1)[:CE] for r in res.results
    ]
    return np.concatenate(parts).reshape(N_PAIRS, 2)


if __name__ == "__main__":
    import os

    pos0 = (
        np.load("/tmp/pos0.npy")
        if os.path.exists("/tmp/pos0.npy")
        else np.array([-1.866805, -0.25733662], np.float32)
    )
    vel0 = (
        np.load("/tmp/vel0.npy")
        if os.path.exists("/tmp/vel0.npy")
        else np.array([-0.847358, -1.5444987], np.float32)
    )
    outv = kernel(np.ones(()), pos0, vel0)
    i = np.arange(N_PAIRS, dtype=np.float64)[:, None]
    closed = (
        pos0.astype(np.float64)
        + i * DT * vel0.astype(np.float64)
        + np.array([0.0, GDT_Y * DT]) * i * (i - 1) / 2.0
    )
    err = np.abs(outv - closed)
    denom = np.maximum(np.abs(closed), 1e-12)
    print("closed-form maxabs-ratio rel err:", err.max() / np.abs(closed).max())
    print("closed-form max elementwise rel err:", (err / denom).max())


# revision 3
# speedup vs baseline: 1.0847x; 1.0847x over previous
"""Trainium2 Bass kernel for nn_BallModel: 10M-step ballistic trajectory.

The reference recurrence (pos += vel*dt; vel += g*dt, recording pos) has the
closed form
    pos_i = pos0 + i*dt*vel0 + g*dt^2 * i*(i-1)/2  =  A + B*i + C*i^2
with A = pos0, B = dt*vel0 - C, C = (g*dt)*dt/2 (per component; C_x = 0).

Output is [10_000_000, 2] f32 (~80 MB), interleaved x,y.  Each of the 8 cores
produces a contiguous 2.5M-element slice (10 MB) -> memory-bound at the
per-core HBM write bandwidth (~358 GB/s line rate => ~28 us drain floor;
the v1 trace showed the drain itself already runs at line rate).

v3 = v2's partition-contiguous layout + HAND-ROLLED synchronization on raw
Bacc (no TileContext).  Rationale, from the v1/v2 traces: TileContext's
epilogue (per-semaphore reset ceremony + two all-engine barriers) burned
~9-10 us AFTER the last output byte, and its preamble event chains delayed
the input DMA ~3 us.  This kernel allocates 5 semaphores and emits exactly
one wait per consumer, so Bacc's legalization fuses every wait into its
consumer instruction and the epilogue is one sync-engine wait plus a 5-sem
clear (for NEFF re-execution safety).

Layout: core-local element index e = p*W + col with W = 19532 (38*512 + 76),
out DRAM [128, W]: each partition owns a contiguous 78,128-byte run of HBM.
128*W = 2,500,096 overlaps 96 elements into the next core's range (host
trims).  Pair index i = e>>1 = Q0 + p*(W/2) + 256*b + jb, where b indexes
the 39 512-column blocks, jb = (col&511)>>1 and col&1 alternate x/y, so ONE
shared rhs table [K=10, 512] drives every block; per-(block,partition)
stationary lhsT tables carry q = Q0 + p*9766 + 256*b.  Values wider than
bf16's 8 mantissa bits are split into 2-3 bf16 rows whose products
accumulate exactly in the fp32 PSUM accumulator (result ~1e-7 rel of the
f64 closed form).

Pipeline: blocks in groups of [1,1,2,4,4,...,4,3] (ramped so the first
output DMA launches after a single matmul + small copy).  Group g: matmuls
-> PSUM pool g%2 ([128,2048] f32, 4 banks) -> copy to the [128,W] SBUF
staging tensor (even groups on vector, odd on scalar, so each consumer
waits on a single semaphore) -> one column-slice output DMA (128
descriptors of up to 8 KB).  All DMAs ride the sync HWDGE queue (gpsimd
SWDGE stalls; scalar HWDGE hard-hangs the device — v1 finding).
"""

import sys
import types

import ml_dtypes
import numpy as np

import concourse.bacc as bacc
import concourse.bass as bass
import concourse.mybir as mybir
from concourse.bass_utils import run_bass_kernel_spmd

# ---- problem constants (hardcoded; kernel.py must be self-contained) ----
N_PAIRS = 10_000_000
ELEMS = 2 * N_PAIRS  # 20,000,000 interleaved f32 values
N_CORES = 8
CE = ELEMS // N_CORES  # 2,500,000 elements per core
P = 128  # partitions
COLS = 512  # one PSUM bank of f32
W = 19532  # elems per partition per core (even; 38*512 + 76)
NB = 39  # column blocks per core (38 full + one 76-col partial)
LAST_BLOCK_COLS = W - 38 * COLS  # 76
K = 10  # matmul contraction rows
GROUP_SIZES = [1, 1, 2, 4, 4, 4, 4, 4, 4, 4, 4, 3]  # sums to 39
HEAD_BLOCKS = 8  # first four groups' lhsT ride the fast head DMA

# fp32-rounded constants, matching the reference's fp32 parameter rounding
DT = float(np.float32(0.01))
GDT_Y = float(np.float32(np.float32(-9.81) * np.float32(0.01)))  # fp32(g_y*dt)
C_Y = GDT_Y * DT / 2.0  # i^2 coefficient for y

_bf16 = ml_dtypes.bfloat16

# exposed for test.py introspection (exec_time_ns etc.)
LAST_RESULTS = None


def _ensure_axon_hooks_stub():
    """bass_utils imports antenv.axon_hooks when BASS_TRACE is set; some
    images lack that module.  Register a stub that degrades to the untraced
    path instead of crashing (test.py replaces it with a real NTFF hook)."""
    try:
        import antenv.axon_hooks  # noqa: F401

        return
    except ImportError:
        pass
    try:
        import antenv  # noqa: F401
    except ImportError:
        return
    stub = types.ModuleType("antenv.axon_hooks")
    stub.get_axon_ntff_profile_hook = lambda: None
    stub.set_axon_ntff_profile_hook = lambda h: None
    sys.modules["antenv.axon_hooks"] = stub


def _build_program() -> bass.Bass:
    # Bacc (not raw Bass): its finalize pipeline runs the sync-wait
    # legalization (fusing our standalone wait_ge's into their consumers)
    # and the register allocation walrus requires.
    nc = bacc.Bacc("TRN2", target_bir_lowering=False)
    hd = nc.declare_dram_parameter(
        "hd", [K, COLS + HEAD_BLOCKS * P], mybir.dt.bfloat16, isOutput=False
    )
    lt_t = nc.declare_dram_parameter(
        "lt_t", [K, (NB - HEAD_BLOCKS) * P], mybir.dt.bfloat16, isOutput=False
    )
    out = nc.declare_dram_parameter("out", [P, W], mybir.dt.float32, isOutput=True)

    hd_s = nc.alloc_sbuf_tensor(
        "hd_s", [K, COLS + HEAD_BLOCKS * P], mybir.dt.bfloat16
    )
    ltt_s = nc.alloc_sbuf_tensor(
        "ltt_s", [K, (NB - HEAD_BLOCKS) * P], mybir.dt.bfloat16
    )
    out_s = nc.alloc_sbuf_tensor("out_s", [P, W], mybir.dt.float32)
    pools = [
        nc.alloc_psum_tensor("pa", [P, 4 * COLS], mybir.dt.float32),
        nc.alloc_psum_tensor("pb", [P, 4 * COLS], mybir.dt.float32),
    ]

    s_in = nc.alloc_semaphore("s_in")  # input DMA completions (+16 each)
    s_pe = nc.alloc_semaphore("s_pe")  # matmul retirements (+1 each)
    s_vc = nc.alloc_semaphore("s_vc")  # vector copy retirements
    s_sc = nc.alloc_semaphore("s_sc")  # scalar copy retirements
    s_do = nc.alloc_semaphore("s_do")  # output DMA completions (+16 each)
    sems = [s_in, s_pe, s_vc, s_sc, s_do]

    nc.sync.dma_start(hd_s[:], hd[:]).then_inc(s_in, 16)
    nc.sync.dma_start(ltt_s[:], lt_t[:]).then_inc(s_in, 16)
    rh_s = hd_s[:, :COLS]

    def lhsT(b):
        if b < HEAD_BLOCKS:
            return hd_s[:, COLS + b * P : COLS + (b + 1) * P]
        b -= HEAD_BLOCKS
        return ltt_s[:, b * P : (b + 1) * P]

    # per-engine copy completion counts: v_done[g]/s_done[g] = value of
    # s_vc/s_sc after group g's copy has retired
    n_groups = len(GROUP_SIZES)
    starts = np.cumsum([0] + GROUP_SIZES).tolist()
    copy_sem = lambda g: s_vc if g % 2 == 0 else s_sc
    copy_val = [0] * n_groups
    cnt = {0: 0, 1: 0}
    for g in range(n_groups):
        cnt[g % 2] += 1
        copy_val[g] = cnt[g % 2]

    used_ltt = False
    for g, nbl in enumerate(GROUP_SIZES):
        b0 = starts[g]
        pt = pools[g % 2]
        # PE gating for this group (waits fuse into the next PE instruction;
        # at most one group needs two, costing a single event-sem instr)
        if g == 0:
            nc.tensor.wait_ge(s_in, 16)  # rh + head lhsT resident
        if not used_ltt and b0 + nbl > HEAD_BLOCKS:
            nc.tensor.wait_ge(s_in, 32)  # lhsT tail resident
            used_ltt = True
        if g >= 2:
            # WAR: pool g%2 was last read by group g-2's copy
            nc.tensor.wait_ge(copy_sem(g - 2), copy_val[g - 2])
        ncols = 0
        for i in range(nbl):
            b = b0 + i
            bc = LAST_BLOCK_COLS if b == NB - 1 else COLS
            nc.tensor.matmul(
                pt[:, i * COLS : i * COLS + bc],
                lhsT(b),
                rh_s[:, :bc],
                start=True,
                stop=True,
            ).then_inc(s_pe, 1)
            ncols = i * COLS + bc
        c0 = b0 * COLS
        eng = nc.vector if g % 2 == 0 else nc.scalar
        eng.wait_ge(s_pe, starts[g] + nbl)
        if g % 2 == 0:
            ci = nc.vector.tensor_copy(out_s[:, c0 : c0 + ncols], pt[:, :ncols])
        else:
            ci = nc.scalar.copy(out_s[:, c0 : c0 + ncols], pt[:, :ncols])
        ci.then_inc(copy_sem(g), 1)
        nc.sync.wait_ge(copy_sem(g), copy_val[g])
        nc.sync.dma_start(out[:, c0 : c0 + ncols], out_s[:, c0 : c0 + ncols]).then_inc(
            s_do, 16
        )

    # Epilogue: kernel completion = all output bytes landed.  gpsimd then
    # re-zeroes our semaphores so a re-execution of this NEFF starts clean
    # (the framework contract is sems == 0 at kernel entry).
    nc.sync.wait_ge(s_do, 16 * n_groups)
    nc.gpsimd.wait_ge(s_do, 16 * n_groups)
    nums = sorted(s.num for s in sems)
    if nums == list(range(nums[0], nums[0] + len(nums))):
        nc.gpsimd.sem_clear(range(nums[0], nums[-1] + 1))
    else:
        for s in sems:
            nc.gpsimd.sem_clear(s)
    nc.finalize()  # runs Bacc.compile(): reg alloc + sync-wait legalization
    return nc


def _split_bf16(x: np.ndarray, n: int):
    """Split x into n bf16 parts summing (nearly) exactly to x."""
    parts = []
    rem = np.asarray(x, dtype=np.float64).copy()
    for _ in range(n):
        p = rem.astype(_bf16)
        parts.append(p)
        rem = rem - p.astype(np.float64)
    return parts


def _host_tables(pos0: np.ndarray, vel0: np.ndarray):
    """Build per-core input tables (float64 math, cast at the end)."""
    ax, ay = float(pos0[0]), float(pos0[1])
    bx_c = DT * float(vel0[0])  # B_x (C_x = 0)
    by_c = DT * float(vel0[1]) - C_Y  # B_y

    # fixed rhs column patterns (jb < 256 within every 512-col block)
    ce = np.arange(COLS)
    j = (ce >> 1).astype(np.float64)
    odd = (ce & 1).astype(np.float64)
    even = 1.0 - odd
    jodd = (j * odd).astype(_bf16)  # exact: j < 256
    resid = np.where(ce & 1 == 1, C_Y * j * j, bx_c * j)
    resid_hi, resid_lo = _split_bf16(resid, 2)
    rh_np = np.stack(
        [
            jodd,
            jodd,
            resid_hi,
            resid_lo,
            odd.astype(_bf16),
            odd.astype(_bf16),
            odd.astype(_bf16),
            even.astype(_bf16),
            even.astype(_bf16),
            even.astype(_bf16),
        ]
    )  # [K, COLS]

    in_maps = []
    b_idx = np.arange(NB, dtype=np.float64)[:, None]  # [NB, 1]
    p_idx = np.arange(P, dtype=np.float64)[None, :]  # [1, P]
    for k in range(N_CORES):
        q = k * (CE // 2) + p_idx * (W // 2) + b_idx * (COLS // 2)  # [NB, P]
        s1_hi, s1_lo = _split_bf16(by_c + 2.0 * C_Y * q, 2)
        ones = np.ones_like(s1_hi)
        by3 = _split_bf16(ay + by_c * q + C_Y * q * q, 3)
        bx3 = _split_bf16(ax + bx_c * q, 3)
        rows = [s1_hi, s1_lo, ones, ones] + by3 + bx3
        lt_np = np.stack([r.reshape(-1) for r in rows])  # [K, NB*P]
        in_maps.append(
            {
                "hd": np.ascontiguousarray(
                    np.concatenate([rh_np, lt_np[:, : HEAD_BLOCKS * P]], axis=1)
                ),
                "lt_t": np.ascontiguousarray(lt_np[:, HEAD_BLOCKS * P :]),
            }
        )
    return in_maps


def kernel(ball_mass, ball_initial_position, ball_initial_velocity) -> np.ndarray:
    global LAST_RESULTS
    pos0 = np.asarray(ball_initial_position, dtype=np.float32)
    vel0 = np.asarray(ball_initial_velocity, dtype=np.float32)

    _ensure_axon_hooks_stub()
    nc = _build_program()
    in_maps = _host_tables(pos0, vel0)
    res = run_bass_kernel_spmd(nc, in_maps, core_ids=list(range(N_CORES)))
    LAST_RESULTS = res

    parts = [
        np.asarray(r["out"], dtype=np.float32).reshape(-1)[:CE] for r in res.results
    ]
    return np.concatenate(parts).reshape(N_PAIRS, 2)


if __name__ == "__main__":
    import os

    pos0 = (
        np.load("/tmp/pos0.npy")
        if os.path.exists("/tmp/pos0.npy")
        else np.array([-1.866805, -0.25733662], np.float32)
    )
    vel0 = (
        np.load("/tmp/vel0.npy")
        if os.path.exists("/tmp/vel0.npy")
        else np.array([-0.847358, -1.5444987], np.float32)
    )
    outv = kernel(np.ones(()), pos0, vel0)
    i = np.arange(N_PAIRS, dtype=np.float64)[:, None]
    closed = (
        pos0.astype(np.float64)
        + i * DT * vel0.astype(np.float64)
        + np.array([0.0, GDT_Y * DT]) * i * (i - 1) / 2.0
    )
    err = np.abs(outv - closed)
    denom = np.maximum(np.abs(closed), 1e-12)
    print("closed-form maxabs-ratio rel err:", err.max() / np.abs(closed).max())
    print("closed-form max elementwise rel err:", (err / denom).max())


# revision 4
# speedup vs baseline: 1.1032x; 1.0171x over previous
"""Trainium2 Bass kernel for nn_BallModel: 10M-step ballistic trajectory.

The reference recurrence (pos += vel*dt; vel += g*dt, recording pos) has the
closed form
    pos_i = pos0 + i*dt*vel0 + g*dt^2 * i*(i-1)/2  =  A + B*i + C*i^2
with A = pos0, B = dt*vel0 - C, C = (g*dt)*dt/2 (per component; C_x = 0).

Output is [10_000_000, 2] f32 (~80 MB), interleaved x,y.  Each of the 8 cores
produces a contiguous 2.5M-element slice (10 MB) -> memory-bound at the
per-core HBM write bandwidth (~358 GB/s line rate => ~28 us drain floor).

v4 = v1's chunk-interleaved, HBM-contiguous layout + HAND-ROLLED
synchronization on raw Bacc (no TileContext).  Trace findings driving this:
 - v1 (TileContext): drain runs at line rate, but the framework epilogue
   (per-semaphore reset ceremony + all-engine barriers) burns ~9 us after
   the last byte and the preamble event chains delay the input DMA.
 - v3 (partition-contiguous layout): raw sync fixed the epilogue, but
   column-slice DMA destinations (128 x 8KB segments at 78 KB stride) drain
   ~10% below line rate; fully-contiguous chunk destinations don't.

Layout (v1's): core element e = c*65536 + p*512 + ce over 39 chunks of
[128 partitions x 512 cols] (one PSUM bank each); chunk c's output region
out[c*128:(c+1)*128, :] is a contiguous 256 KB of HBM.  Pair index
i = q + jb with q(c,p) = core*1.25e6 + c*32768 + p*256, jb = ce>>1, and
ce&1 alternating x/y, so one shared rhs table [K=10, 512] drives every
chunk; per-(chunk,partition) stationary lhsT tables carry q.  Values wider
than bf16's 8 mantissa bits are split into 2-3 bf16 rows whose products
accumulate exactly in the fp32 PSUM accumulator (result ~1e-7 rel of the
f64 closed form).

Pipeline: chunks in groups of [1,1,2,4,4,...,4,3] (ramped so the first
output DMA launches after a single matmul + small copy).  Group g: matmuls
-> PSUM pool g%2 ([128,2048] f32, 4 banks) -> one copy into the [128,19968]
SBUF staging tensor (even groups on vector, odd on scalar, so every
consumer waits on a single semaphore that Bacc fuses into the consuming
instruction) -> one output DMA (multi-chunk groups use the p<->j rearrange
so the destination stays one contiguous HBM run; measured line rate in v1).
The partial last chunk (19 useful rows) ships as a separate tiny final DMA
so the drain ends on a fast completion.  5 semaphores total; the epilogue
is one sync-engine wait plus a 5-sem clear for NEFF re-execution safety.
All DMAs ride the sync HWDGE queue (gpsimd SWDGE stalls; scalar HWDGE
hard-hangs the device — v1 finding).
"""

import sys
import types

import ml_dtypes
import numpy as np

import concourse.bacc as bacc
import concourse.bass as bass
import concourse.mybir as mybir
from concourse.bass_utils import run_bass_kernel_spmd

# ---- problem constants (hardcoded; kernel.py must be self-contained) ----
N_PAIRS = 10_000_000
ELEMS = 2 * N_PAIRS  # 20,000,000 interleaved f32 values
N_CORES = 8
CE = ELEMS // N_CORES  # 2,500,000 elements per core
P = 128  # partitions
COLS = 512  # one PSUM bank of f32
CHUNK = P * COLS  # 65,536 elements per chunk
NB = 39  # chunks per core (38 full + one partial)
LAST_ROWS = -(-(CE - 38 * CHUNK) // COLS)  # 19 useful rows of final chunk
K = 10  # matmul contraction rows
GROUP_SIZES = [1, 1, 2, 4, 4, 4, 4, 4, 4, 4, 4, 3]  # sums to 39
HEAD_BLOCKS = 8  # first four groups' lhsT ride the fast head DMA

# fp32-rounded constants, matching the reference's fp32 parameter rounding
DT = float(np.float32(0.01))
GDT_Y = float(np.float32(np.float32(-9.81) * np.float32(0.01)))  # fp32(g_y*dt)
C_Y = GDT_Y * DT / 2.0  # i^2 coefficient for y

_bf16 = ml_dtypes.bfloat16

# exposed for test.py introspection (exec_time_ns etc.)
LAST_RESULTS = None


def _ensure_axon_hooks_stub():
    """bass_utils imports antenv.axon_hooks when BASS_TRACE is set; some
    images lack that module.  Register a stub that degrades to the untraced
    path instead of crashing (test.py replaces it with a real NTFF hook)."""
    try:
        import antenv.axon_hooks  # noqa: F401

        return
    except ImportError:
        pass
    try:
        import antenv  # noqa: F401
    except ImportError:
        return
    stub = types.ModuleType("antenv.axon_hooks")
    stub.get_axon_ntff_profile_hook = lambda: None
    stub.set_axon_ntff_profile_hook = lambda h: None
    sys.modules["antenv.axon_hooks"] = stub


def _build_program() -> bass.Bass:
    # Bacc (not raw Bass): its finalize pipeline runs the sync-wait
    # legalization (fusing our standalone wait_ge's into their consumers)
    # and the register allocation walrus requires.
    nc = bacc.Bacc("TRN2", target_bir_lowering=False)
    hd = nc.declare_dram_parameter(
        "hd", [K, COLS + HEAD_BLOCKS * P], mybir.dt.bfloat16, isOutput=False
    )
    lt_t = nc.declare_dram_parameter(
        "lt_t", [K, (NB - HEAD_BLOCKS) * P], mybir.dt.bfloat16, isOutput=False
    )
    out = nc.declare_dram_parameter(
        "out", [NB * P, COLS], mybir.dt.float32, isOutput=True
    )

    hd_s = nc.alloc_sbuf_tensor(
        "hd_s", [K, COLS + HEAD_BLOCKS * P], mybir.dt.bfloat16
    )
    ltt_s = nc.alloc_sbuf_tensor(
        "ltt_s", [K, (NB - HEAD_BLOCKS) * P], mybir.dt.bfloat16
    )
    out_s = nc.alloc_sbuf_tensor("out_s", [P, NB * COLS], mybir.dt.float32)
    pools = [
        nc.alloc_psum_tensor("pa", [P, 4 * COLS], mybir.dt.float32),
        nc.alloc_psum_tensor("pb", [P, 4 * COLS], mybir.dt.float32),
    ]

    s_in = nc.alloc_semaphore("s_in")  # input DMA completions (+16 each)
    s_pe = nc.alloc_semaphore("s_pe")  # matmul retirements (+1 each)
    s_vc = nc.alloc_semaphore("s_vc")  # vector copy retirements
    s_sc = nc.alloc_semaphore("s_sc")  # scalar copy retirements
    s_do = nc.alloc_semaphore("s_do")  # output DMA completions (+16 each)
    sems = [s_in, s_pe, s_vc, s_sc, s_do]

    nc.sync.dma_start(hd_s[:], hd[:]).then_inc(s_in, 16)
    nc.sync.dma_start(ltt_s[:], lt_t[:]).then_inc(s_in, 16)
    rh_s = hd_s[:, :COLS]

    def lhsT(b):
        if b < HEAD_BLOCKS:
            return hd_s[:, COLS + b * P : COLS + (b + 1) * P]
        b -= HEAD_BLOCKS
        return ltt_s[:, b * P : (b + 1) * P]

    n_groups = len(GROUP_SIZES)
    starts = np.cumsum([0] + GROUP_SIZES).tolist()
    copy_sem = lambda g: s_vc if g % 2 == 0 else s_sc
    copy_val = [0] * n_groups
    cnt = {0: 0, 1: 0}
    for g in range(n_groups):
        cnt[g % 2] += 1
        copy_val[g] = cnt[g % 2]

    n_dma = 0
    used_ltt = False
    for g, nbl in enumerate(GROUP_SIZES):
        b0 = starts[g]
        pt = pools[g % 2]
        # PE gating (waits fuse into the next PE instruction; at most one
        # group carries two waits, costing a single event-sem instruction)
        if g == 0:
            nc.tensor.wait_ge(s_in, 16)  # rh + head lhsT resident
        if not used_ltt and b0 + nbl > HEAD_BLOCKS:
            nc.tensor.wait_ge(s_in, 32)  # lhsT tail resident
            used_ltt = True
        if g >= 2:
            # WAR: pool g%2 was last read by group g-2's copy
            nc.tensor.wait_ge(copy_sem(g - 2), copy_val[g - 2])
        for i in range(nbl):
            nc.tensor.matmul(
                pt[:, i * COLS : (i + 1) * COLS],
                lhsT(b0 + i),
                rh_s,
                start=True,
                stop=True,
            ).then_inc(s_pe, 1)
        ncols = nbl * COLS
        c0 = b0 * COLS
        eng = nc.vector if g % 2 == 0 else nc.scalar
        eng.wait_ge(s_pe, starts[g] + nbl)
        if g % 2 == 0:
            ci = nc.vector.tensor_copy(out_s[:, c0 : c0 + ncols], pt[:, :ncols])
        else:
            ci = nc.scalar.copy(out_s[:, c0 : c0 + ncols], pt[:, :ncols])
        ci.then_inc(copy_sem(g), 1)
        # output DMA(s) for this group; multi-chunk groups rearrange so the
        # HBM destination is one contiguous run.  The partial final chunk
        # (NB-1) ships separately, trimmed to its useful rows, and LAST so
        # the drain ends on a fast small completion.
        full = nbl - 1 if b0 + nbl == NB else nbl
        nc.sync.wait_ge(copy_sem(g), copy_val[g])
        if full == 1:
            dst = out[b0 * P : (b0 + 1) * P, :]
            src = out_s[:, c0 : c0 + COLS]
        else:
            dst = out[b0 * P : (b0 + full) * P, :].rearrange(
                "(j p) q -> p j q", p=P
            )
            src = out_s[:, c0 : c0 + full * COLS].rearrange(
                "p (j q) -> p j q", q=COLS
            )
        nc.sync.dma_start(dst, src).then_inc(s_do, 16)
        n_dma += 1
        if full != nbl:  # trimmed partial final chunk
            b = b0 + full
            dst = out[b * P : b * P + LAST_ROWS, :]
            src = out_s[:LAST_ROWS, b * COLS : (b + 1) * COLS]
            nc.sync.dma_start(dst, src).then_inc(s_do, 16)
            n_dma += 1

    # Epilogue: kernel completion = all output bytes landed.  gpsimd then
    # re-zeroes our semaphores so a re-execution of this NEFF starts clean
    # (the framework contract is sems == 0 at kernel entry).
    nc.sync.wait_ge(s_do, 16 * n_dma)
    nc.gpsimd.wait_ge(s_do, 16 * n_dma)
    nums = sorted(s.num for s in sems)
    if nums == list(range(nums[0], nums[0] + len(nums))):
        nc.gpsimd.sem_clear(range(nums[0], nums[-1] + 1))
    else:
        for s in sems:
            nc.gpsimd.sem_clear(s)
    nc.finalize()  # runs Bacc.compile(): reg alloc + sync-wait legalization
    return nc


def _split_bf16(x: np.ndarray, n: int):
    """Split x into n bf16 parts summing (nearly) exactly to x."""
    parts = []
    rem = np.asarray(x, dtype=np.float64).copy()
    for _ in range(n):
        p = rem.astype(_bf16)
        parts.append(p)
        rem = rem - p.astype(np.float64)
    return parts


def _host_tables(pos0: np.ndarray, vel0: np.ndarray):
    """Build per-core input tables (float64 math, cast at the end)."""
    ax, ay = float(pos0[0]), float(pos0[1])
    bx_c = DT * float(vel0[0])  # B_x (C_x = 0)
    by_c = DT * float(vel0[1]) - C_Y  # B_y

    # fixed rhs column patterns (jb < 256 within every 512-col chunk)
    ce = np.arange(COLS)
    j = (ce >> 1).astype(np.float64)
    odd = (ce & 1).astype(np.float64)
    even = 1.0 - odd
    jodd = (j * odd).astype(_bf16)  # exact: j < 256
    resid = np.where(ce & 1 == 1, C_Y * j * j, bx_c * j)
    resid_hi, resid_lo = _split_bf16(resid, 2)
    rh_np = np.stack(
        [
            jodd,
            jodd,
            resid_hi,
            resid_lo,
            odd.astype(_bf16),
            odd.astype(_bf16),
            odd.astype(_bf16),
            even.astype(_bf16),
            even.astype(_bf16),
            even.astype(_bf16),
        ]
    )  # [K, COLS]

    in_maps = []
    c_idx = np.arange(NB, dtype=np.float64)[:, None]  # [NB, 1]
    p_idx = np.arange(P, dtype=np.float64)[None, :]  # [1, P]
    for k in range(N_CORES):
        q = k * (CE // 2) + c_idx * (CHUNK // 2) + p_idx * (COLS // 2)  # [NB, P]
        s1_hi, s1_lo = _split_bf16(by_c + 2.0 * C_Y * q, 2)
        ones = np.ones_like(s1_hi)
        by3 = _split_bf16(ay + by_c * q + C_Y * q * q, 3)
        bx3 = _split_bf16(ax + bx_c * q, 3)
        rows = [s1_hi, s1_lo, ones, ones] + by3 + bx3
        lt_np = np.stack([r.reshape(-1) for r in rows])  # [K, NB*P]
        in_maps.append(
            {
                "hd": np.ascontiguousarray(
                    np.concatenate([rh_np, lt_np[:, : HEAD_BLOCKS * P]], axis=1)
                ),
                "lt_t": np.ascontiguousarray(lt_np[:, HEAD_BLOCKS * P :]),
            }
        )
    return in_maps


def kernel(ball_mass, ball_initial_position, ball_initial_velocity) -> np.ndarray:
    global LAST_RESULTS
    pos0 = np.asarray(ball_initial_position, dtype=np.float32)
    vel0 = np.asarray(ball_initial_velocity, dtype=np.float32)

    _ensure_axon_hooks_stub()
    nc = _build_program()
    in_maps = _host_tables(pos0, vel0)
    res = run_bass_kernel_spmd(nc, in_maps, core_ids=list(range(N_CORES)))
    LAST_RESULTS = res

    parts = [
        np.asarray(r["out"], dtype=np.float32).reshape(-1)[:CE] for r in res.results
    ]
    return np.concatenate(parts).reshape(N_PAIRS, 2)


if __name__ == "__main__":
    import os

    pos0 = (
        np.load("/tmp/pos0.npy")
        if os.path.exists("/tmp/pos0.npy")
        else np.array([-1.866805, -0.25733662], np.float32)
    )
    vel0 = (
        np.load("/tmp/vel0.npy")
        if os.path.exists("/tmp/vel0.npy")
        else np.array([-0.847358, -1.5444987], np.float32)
    )
    outv = kernel(np.ones(()), pos0, vel0)
    i = np.arange(N_PAIRS, dtype=np.float64)[:, None]
    closed = (
        pos0.astype(np.float64)
        + i * DT * vel0.astype(np.float64)
        + np.array([0.0, GDT_Y * DT]) * i * (i - 1) / 2.0
    )
    err = np.abs(outv - closed)
    denom = np.maximum(np.abs(closed), 1e-12)
    print("closed-form maxabs-ratio rel err:", err.max() / np.abs(closed).max())
    print("closed-form max elementwise rel err:", (err / denom).max())


# revision 7
# speedup vs baseline: 1.1175x; 1.0130x over previous
"""Trainium2 Bass kernel for nn_BallModel: 10M-step ballistic trajectory.

The reference recurrence (pos += vel*dt; vel += g*dt, recording pos) has the
closed form
    pos_i = pos0 + i*dt*vel0 + g*dt^2 * i*(i-1)/2  =  A + B*i + C*i^2
with A = pos0, B = dt*vel0 - C, C = (g*dt)*dt/2 (per component; C_x = 0).

Output is [10_000_000, 2] f32 (~80 MB), interleaved x,y.  Each of the 8 cores
produces a contiguous 2.5M-element slice (10 MB) -> memory-bound at the
per-core HBM write bandwidth (~358 GB/s line rate => ~28 us drain floor).

v4 = v1's chunk-interleaved, HBM-contiguous layout + HAND-ROLLED
synchronization on raw Bacc (no TileContext).  Trace findings driving this:
 - v1 (TileContext): drain runs at line rate, but the framework epilogue
   (per-semaphore reset ceremony + all-engine barriers) burns ~9 us after
   the last byte and the preamble event chains delay the input DMA.
 - v3 (partition-contiguous layout): raw sync fixed the epilogue, but
   column-slice DMA destinations (128 x 8KB segments at 78 KB stride) drain
   ~10% below line rate; fully-contiguous chunk destinations don't.

Layout (v1's): core element e = c*65536 + p*512 + ce over 39 chunks of
[128 partitions x 512 cols] (one PSUM bank each); chunk c's output region
out[c*128:(c+1)*128, :] is a contiguous 256 KB of HBM.  Pair index
i = q + jb with q(c,p) = core*1.25e6 + c*32768 + p*256, jb = ce>>1, and
ce&1 alternating x/y, so one shared rhs table [K=10, 512] drives every
chunk; per-(chunk,partition) stationary lhsT tables carry q.  Values wider
than bf16's 8 mantissa bits are split into 2-3 bf16 rows whose products
accumulate exactly in the fp32 PSUM accumulator (result ~1e-7 rel of the
f64 closed form).

Pipeline: chunks in groups of [1,1,2,4,4,...,4,3] (ramped so the first
output DMA launches after a single matmul + small copy).  Group g: matmuls
-> PSUM pool g%2 ([128,2048] f32, 4 banks) -> one copy into the [128,19968]
SBUF staging tensor (even groups on vector, odd on scalar, so every
consumer waits on a single semaphore that Bacc fuses into the consuming
instruction) -> one output DMA (multi-chunk groups use the p<->j rearrange
so the destination stays one contiguous HBM run; measured line rate in v1).
The partial last chunk (19 useful rows) ships as a separate tiny final DMA
so the drain ends on a fast completion.  5 semaphores total; the epilogue
is one sync-engine wait plus a 5-sem clear for NEFF re-execution safety.
All DMAs ride the sync HWDGE queue (gpsimd SWDGE stalls; scalar HWDGE
hard-hangs the device — v1 finding).
"""

import sys
import types

import ml_dtypes
import numpy as np

import concourse.bacc as bacc
import concourse.bass as bass
import concourse.mybir as mybir
from concourse.bass_utils import run_bass_kernel_spmd

# ---- problem constants (hardcoded; kernel.py must be self-contained) ----
N_PAIRS = 10_000_000
ELEMS = 2 * N_PAIRS  # 20,000,000 interleaved f32 values
N_CORES = 8
CE = ELEMS // N_CORES  # 2,500,000 elements per core
P = 128  # partitions
COLS = 512  # one PSUM bank of f32
CHUNK = P * COLS  # 65,536 elements per chunk
NB = 39  # chunks per core (38 full + one partial)
LAST_ROWS = -(-(CE - 38 * CHUNK) // COLS)  # 19 useful rows of final chunk
K = 10  # matmul contraction rows
GROUP_SIZES = [1] * 8 + [4] * 7 + [3]  # sums to 39
HEAD_BLOCKS = 8  # the 8 single-chunk ramp groups' lhsT ride the fast head DMA

# fp32-rounded constants, matching the reference's fp32 parameter rounding
DT = float(np.float32(0.01))
GDT_Y = float(np.float32(np.float32(-9.81) * np.float32(0.01)))  # fp32(g_y*dt)
C_Y = GDT_Y * DT / 2.0  # i^2 coefficient for y

_bf16 = ml_dtypes.bfloat16

# exposed for test.py introspection (exec_time_ns etc.)
LAST_RESULTS = None


def _ensure_axon_hooks_stub():
    """bass_utils imports antenv.axon_hooks when BASS_TRACE is set; some
    images lack that module.  Register a stub that degrades to the untraced
    path instead of crashing (test.py replaces it with a real NTFF hook)."""
    try:
        import antenv.axon_hooks  # noqa: F401

        return
    except ImportError:
        pass
    try:
        import antenv  # noqa: F401
    except ImportError:
        return
    stub = types.ModuleType("antenv.axon_hooks")
    stub.get_axon_ntff_profile_hook = lambda: None
    stub.set_axon_ntff_profile_hook = lambda h: None
    sys.modules["antenv.axon_hooks"] = stub


def _build_program() -> bass.Bass:
    # Bacc (not raw Bass): its finalize pipeline runs the sync-wait
    # legalization (fusing our standalone wait_ge's into their consumers)
    # and the register allocation walrus requires.
    nc = bacc.Bacc("TRN2", target_bir_lowering=False)
    hd = nc.declare_dram_parameter(
        "hd", [K, COLS + HEAD_BLOCKS * P], mybir.dt.bfloat16, isOutput=False
    )
    lt_t = nc.declare_dram_parameter(
        "lt_t", [K, (NB - HEAD_BLOCKS) * P], mybir.dt.bfloat16, isOutput=False
    )
    out = nc.declare_dram_parameter(
        "out", [NB * P, COLS], mybir.dt.float32, isOutput=True
    )

    hd_s = nc.alloc_sbuf_tensor(
        "hd_s", [K, COLS + HEAD_BLOCKS * P], mybir.dt.bfloat16
    )
    ltt_s = nc.alloc_sbuf_tensor(
        "ltt_s", [K, (NB - HEAD_BLOCKS) * P], mybir.dt.bfloat16
    )
    # one dedicated staging tile per group: v1/v4 A/B showed the descriptor
    # generator emits contiguous-2KB-per-destination descriptors (line-rate
    # drain) for standalone tiles, but 8KB strided-destination descriptors
    # (~10% slower) when the source is a column slice of one big tensor
    ot_s = [
        nc.alloc_sbuf_tensor(f"ot{g}", [P, n * COLS], mybir.dt.float32)
        for g, n in enumerate(GROUP_SIZES)
    ]
    pools = [
        nc.alloc_psum_tensor("pa", [P, 4 * COLS], mybir.dt.float32),
        nc.alloc_psum_tensor("pb", [P, 4 * COLS], mybir.dt.float32),
    ]

    s_in = nc.alloc_semaphore("s_in")  # input DMA completions (+16 each)
    s_pe = nc.alloc_semaphore("s_pe")  # matmul retirements (+1 each)
    s_vc = nc.alloc_semaphore("s_vc")  # vector copy retirements
    s_sc = nc.alloc_semaphore("s_sc")  # scalar copy retirements
    s_do = nc.alloc_semaphore("s_do")  # output DMA completions (+16 each)
    sems = [s_in, s_pe, s_vc, s_sc, s_do]

    nc.sync.dma_start(hd_s[:], hd[:]).then_inc(s_in, 16)
    nc.sync.dma_start(ltt_s[:], lt_t[:]).then_inc(s_in, 16)
    rh_s = hd_s[:, :COLS]

    def lhsT(b):
        if b < HEAD_BLOCKS:
            return hd_s[:, COLS + b * P : COLS + (b + 1) * P]
        b -= HEAD_BLOCKS
        return ltt_s[:, b * P : (b + 1) * P]

    n_groups = len(GROUP_SIZES)
    starts = np.cumsum([0] + GROUP_SIZES).tolist()
    copy_sem = lambda g: s_vc if g % 2 == 0 else s_sc
    copy_val = [0] * n_groups
    cnt = {0: 0, 1: 0}
    for g in range(n_groups):
        cnt[g % 2] += 1
        copy_val[g] = cnt[g % 2]

    n_ramp = sum(1 for n in GROUP_SIZES if n == 1)  # leading single-chunk groups
    n_dma = 0
    used_ltt = False
    for g, nbl in enumerate(GROUP_SIZES):
        b0 = starts[g]
        # ramp singles park in distinct bank quarters of the two pools so
        # they need no WAR waits at all; full groups use a whole pool
        if nbl == 1:
            pt = pools[g % 2][:, (g // 2) * COLS : (g // 2 + 1) * COLS]
        else:
            pt = pools[g % 2]
        # PE gating (waits fuse into the next PE instruction; at most one
        # group carries two waits, costing a single event-sem instruction)
        if g == 0:
            nc.tensor.wait_ge(s_in, 16)  # rh + head lhsT resident
        if not used_ltt and b0 + nbl > HEAD_BLOCKS:
            nc.tensor.wait_ge(s_in, 32)  # lhsT tail resident
            used_ltt = True
        if g == n_ramp or g == n_ramp + 1:
            # WAR: the full pool was last read by the same-parity ramp copies
            nc.tensor.wait_ge(copy_sem(g), copy_val[g] - 1)
        elif g >= n_ramp + 2:
            # WAR: pool g%2 was last read by group g-2's copy
            nc.tensor.wait_ge(copy_sem(g - 2), copy_val[g - 2])
        for i in range(nbl):
            nc.tensor.matmul(
                pt[:, i * COLS : (i + 1) * COLS],
                lhsT(b0 + i),
                rh_s,
                start=True,
                stop=True,
            ).then_inc(s_pe, 1)
        ncols = nbl * COLS
        ot = ot_s[g]
        eng = nc.vector if g % 2 == 0 else nc.scalar
        eng.wait_ge(s_pe, starts[g] + nbl)
        if g % 2 == 0:
            ci = nc.vector.tensor_copy(ot[:], pt[:, :ncols])
        else:
            ci = nc.scalar.copy(ot[:], pt[:, :ncols])
        ci.then_inc(copy_sem(g), 1)
        # output DMA(s) for this group; multi-chunk groups rearrange so the
        # HBM destination is one contiguous run.  The partial final chunk
        # (NB-1) ships separately, trimmed to its useful rows, and LAST so
        # the drain ends on a fast small completion.
        full = nbl - 1 if b0 + nbl == NB else nbl
        nc.sync.wait_ge(copy_sem(g), copy_val[g])
        if full == 1:
            dst = out[b0 * P : (b0 + 1) * P, :]
            src = ot[:, :COLS]
        else:
            dst = out[b0 * P : (b0 + full) * P, :].rearrange(
                "(j p) q -> p j q", p=P
            )
            src = ot[:, : full * COLS].rearrange("p (j q) -> p j q", q=COLS)
        nc.sync.dma_start(dst, src).then_inc(s_do, 16)
        n_dma += 1
        if full != nbl:  # trimmed partial final chunk
            b = b0 + full
            dst = out[b * P : b * P + LAST_ROWS, :]
            src = ot[:LAST_ROWS, full * COLS : (full + 1) * COLS]
            nc.sync.dma_start(dst, src).then_inc(s_do, 16)
            n_dma += 1

    # Epilogue: kernel completion = all output bytes landed.  gpsimd then
    # re-zeroes our semaphores so a re-execution of this NEFF starts clean
    # (the framework contract is sems == 0 at kernel entry).
    nc.sync.wait_ge(s_do, 16 * n_dma)
    nc.gpsimd.wait_ge(s_do, 16 * n_dma)
    nums = sorted(s.num for s in sems)
    if nums == list(range(nums[0], nums[0] + len(nums))):
        nc.gpsimd.sem_clear(range(nums[0], nums[-1] + 1))
    else:
        for s in sems:
            nc.gpsimd.sem_clear(s)
    nc.finalize()  # runs Bacc.compile(): reg alloc + sync-wait legalization
    return nc


def _split_bf16(x: np.ndarray, n: int):
    """Split x into n bf16 parts summing (nearly) exactly to x."""
    parts = []
    rem = np.asarray(x, dtype=np.float64).copy()
    for _ in range(n):
        p = rem.astype(_bf16)
        parts.append(p)
        rem = rem - p.astype(np.float64)
    return parts


def _host_tables(pos0: np.ndarray, vel0: np.ndarray):
    """Build per-core input tables (float64 math, cast at the end)."""
    ax, ay = float(pos0[0]), float(pos0[1])
    bx_c = DT * float(vel0[0])  # B_x (C_x = 0)
    by_c = DT * float(vel0[1]) - C_Y  # B_y

    # fixed rhs column patterns (jb < 256 within every 512-col chunk)
    ce = np.arange(COLS)
    j = (ce >> 1).astype(np.float64)
    odd = (ce & 1).astype(np.float64)
    even = 1.0 - odd
    jodd = (j * odd).astype(_bf16)  # exact: j < 256
    resid = np.where(ce & 1 == 1, C_Y * j * j, bx_c * j)
    resid_hi, resid_lo = _split_bf16(resid, 2)
    rh_np = np.stack(
        [
            jodd,
            jodd,
            resid_hi,
            resid_lo,
            odd.astype(_bf16),
            odd.astype(_bf16),
            odd.astype(_bf16),
            even.astype(_bf16),
            even.astype(_bf16),
            even.astype(_bf16),
        ]
    )  # [K, COLS]

    in_maps = []
    c_idx = np.arange(NB, dtype=np.float64)[:, None]  # [NB, 1]
    p_idx = np.arange(P, dtype=np.float64)[None, :]  # [1, P]
    for k in range(N_CORES):
        q = k * (CE // 2) + c_idx * (CHUNK // 2) + p_idx * (COLS // 2)  # [NB, P]
        s1_hi, s1_lo = _split_bf16(by_c + 2.0 * C_Y * q, 2)
        ones = np.ones_like(s1_hi)
        by3 = _split_bf16(ay + by_c * q + C_Y * q * q, 3)
        bx3 = _split_bf16(ax + bx_c * q, 3)
        rows = [s1_hi, s1_lo, ones, ones] + by3 + bx3
        lt_np = np.stack([r.reshape(-1) for r in rows])  # [K, NB*P]
        in_maps.append(
            {
                "hd": np.ascontiguousarray(
                    np.concatenate([rh_np, lt_np[:, : HEAD_BLOCKS * P]], axis=1)
                ),
                "lt_t": np.ascontiguousarray(lt_np[:, HEAD_BLOCKS * P :]),
            }
        )
    return in_maps


def kernel(ball_mass, ball_initial_position, ball_initial_velocity) -> np.ndarray:
    global LAST_RESULTS
    pos0 = np.asarray(ball_initial_position, dtype=np.float32)
    vel0 = np.asarray(ball_initial_velocity, dtype=np.float32)

    _ensure_axon_hooks_stub()
    nc = _build_program()
    in_maps = _host_tables(pos0, vel0)
    res = run_bass_kernel_spmd(nc, in_maps, core_ids=list(range(N_CORES)))
    LAST_RESULTS = res

    parts = [
        np.asarray(r["out"], dtype=np.float32).reshape(-1)[:CE] for r in res.results
    ]
    return np.concatenate(parts).reshape(N_PAIRS, 2)


if __name__ == "__main__":
    import os

    pos0 = (
        np.load("/tmp/pos0.npy")
        if os.path.exists("/tmp/pos0.npy")
        else np.array([-1.866805, -0.25733662], np.float32)
    )
    vel0 = (
        np.load("/tmp/vel0.npy")
        if os.path.exists("/tmp/vel0.npy")
        else np.array([-0.847358, -1.5444987], np.float32)
    )
    outv = kernel(np.ones(()), pos0, vel0)
    i = np.arange(N_PAIRS, dtype=np.float64)[:, None]
    closed = (
        pos0.astype(np.float64)
        + i * DT * vel0.astype(np.float64)
        + np.array([0.0, GDT_Y * DT]) * i * (i - 1) / 2.0
    )
    err = np.abs(outv - closed)
    denom = np.maximum(np.abs(closed), 1e-12)
    print("closed-form maxabs-ratio rel err:", err.max() / np.abs(closed).max())
    print("closed-form max elementwise rel err:", (err / denom).max())


# revision 11
# speedup vs baseline: 1.2120x; 1.0846x over previous
"""Trainium2 Bass kernel for nn_BallModel: 10M-step ballistic trajectory.

The reference recurrence (pos += vel*dt; vel += g*dt, recording pos) has the
closed form
    pos_i = pos0 + i*dt*vel0 + g*dt^2 * i*(i-1)/2  =  A + B*i + C*i^2
with A = pos0, B = dt*vel0 - C, C = (g*dt)*dt/2 (per component; C_x = 0).

Output is [10_000_000, 2] f32 (~80 MB), interleaved x,y.  Each of the 8 cores
produces a contiguous 2.5M-element slice (10 MB) -> memory-bound at the
per-core HBM write bandwidth (~358 GB/s line rate => ~28 us drain floor).

v4 = v1's chunk-interleaved, HBM-contiguous layout + HAND-ROLLED
synchronization on raw Bacc (no TileContext).  Trace findings driving this:
 - v1 (TileContext): drain runs at line rate, but the framework epilogue
   (per-semaphore reset ceremony + all-engine barriers) burns ~9 us after
   the last byte and the preamble event chains delay the input DMA.
 - v3 (partition-contiguous layout): raw sync fixed the epilogue, but
   column-slice DMA destinations (128 x 8KB segments at 78 KB stride) drain
   ~10% below line rate; fully-contiguous chunk destinations don't.

Layout (v1's): core element e = c*65536 + p*512 + ce over 39 chunks of
[128 partitions x 512 cols] (one PSUM bank each); chunk c's output region
out[c*128:(c+1)*128, :] is a contiguous 256 KB of HBM.  Pair index
i = q + jb with q(c,p) = core*1.25e6 + c*32768 + p*256, jb = ce>>1, and
ce&1 alternating x/y, so one shared rhs table [K=10, 512] drives every
chunk; per-(chunk,partition) stationary lhsT tables carry q.  Values wider
than bf16's 8 mantissa bits are split into 2-3 bf16 rows whose products
accumulate exactly in the fp32 PSUM accumulator (result ~1e-7 rel of the
f64 closed form).

Pipeline: chunks in groups of [1,1,2,4,4,...,4,3] (ramped so the first
output DMA launches after a single matmul + small copy).  Group g: matmuls
-> PSUM pool g%2 ([128,2048] f32, 4 banks) -> one copy into the [128,19968]
SBUF staging tensor (even groups on vector, odd on scalar, so every
consumer waits on a single semaphore that Bacc fuses into the consuming
instruction) -> one output DMA (multi-chunk groups use the p<->j rearrange
so the destination stays one contiguous HBM run; measured line rate in v1).
The partial last chunk (19 useful rows) ships as a separate tiny final DMA
so the drain ends on a fast completion.  5 semaphores total; the epilogue
is one sync-engine wait plus a 5-sem clear for NEFF re-execution safety.
All DMAs ride the sync HWDGE queue (gpsimd SWDGE stalls; scalar HWDGE
hard-hangs the device — v1 finding).
"""

import sys
import types

import ml_dtypes
import numpy as np

import concourse.bacc as bacc
import concourse.bass as bass
import concourse.mybir as mybir
from concourse.bass_utils import run_bass_kernel_spmd

# ---- problem constants (hardcoded; kernel.py must be self-contained) ----
N_PAIRS = 10_000_000
ELEMS = 2 * N_PAIRS  # 20,000,000 interleaved f32 values
N_CORES = 8
CE = ELEMS // N_CORES  # 2,500,000 elements per core
P = 128  # partitions
COLS = 512  # one PSUM bank of f32
CHUNK = P * COLS  # 65,536 elements per chunk
NB = 39  # chunks per core (38 full + one partial)
LAST_ROWS = -(-(CE - 38 * CHUNK) // COLS)  # 19 useful rows of final chunk
K = 10  # matmul contraction rows
# group schedule over chunks, by chunk index.  The tiny partial chunk (38)
# ships FIRST (its copy is 19 rows, its DMA 38 KB) so the drain never ends
# on a slow straggler; then 8 single-chunk ramp groups (first output DMA
# ~1.3 us after the first matmul); then 2-chunk groups whose ~1.1 us copy
# latency keeps supply (~500 B/ns) above the ~358 B/ns drain with no
# transition stall (4-chunk copies at 2.3 us caused one).
GROUPS = [[38]] + [[c] for c in range(8)] + [[c, c + 1] for c in range(8, 38, 2)]
RAMP_CHUNKS = [38] + list(range(8))  # chunks whose lhsT rides the head DMA
HEAD_BLOCKS = len(RAMP_CHUNKS)  # 9

# fp32-rounded constants, matching the reference's fp32 parameter rounding
DT = float(np.float32(0.01))
GDT_Y = float(np.float32(np.float32(-9.81) * np.float32(0.01)))  # fp32(g_y*dt)
C_Y = GDT_Y * DT / 2.0  # i^2 coefficient for y

_bf16 = ml_dtypes.bfloat16

# exposed for test.py introspection (exec_time_ns etc.)
LAST_RESULTS = None


def _ensure_axon_hooks_stub():
    """bass_utils imports antenv.axon_hooks when BASS_TRACE is set; some
    images lack that module.  Register a stub that degrades to the untraced
    path instead of crashing (test.py replaces it with a real NTFF hook)."""
    try:
        import antenv.axon_hooks  # noqa: F401

        return
    except ImportError:
        pass
    try:
        import antenv  # noqa: F401
    except ImportError:
        return
    stub = types.ModuleType("antenv.axon_hooks")
    stub.get_axon_ntff_profile_hook = lambda: None
    stub.set_axon_ntff_profile_hook = lambda h: None
    sys.modules["antenv.axon_hooks"] = stub


def _build_program() -> bass.Bass:
    # Bacc (not raw Bass): its finalize pipeline runs the sync-wait
    # legalization (fusing our standalone wait_ge's into their consumers)
    # and the register allocation walrus requires.
    nc = bacc.Bacc("TRN2", target_bir_lowering=False)
    hd = nc.declare_dram_parameter(
        "hd", [K, COLS + HEAD_BLOCKS * P], mybir.dt.bfloat16, isOutput=False
    )
    lt_t = nc.declare_dram_parameter(
        "lt_t", [K, (NB - HEAD_BLOCKS) * P], mybir.dt.bfloat16, isOutput=False
    )
    out = nc.declare_dram_parameter(
        "out", [NB * P, COLS], mybir.dt.float32, isOutput=True
    )

    hd_s = nc.alloc_sbuf_tensor(
        "hd_s", [K, COLS + HEAD_BLOCKS * P], mybir.dt.bfloat16
    )
    ltt_s = nc.alloc_sbuf_tensor(
        "ltt_s", [K, (NB - HEAD_BLOCKS) * P], mybir.dt.bfloat16
    )
    # one dedicated staging tile per group: v1/v4 A/B showed the descriptor
    # generator emits contiguous-2KB-per-destination descriptors (line-rate
    # drain) for standalone tiles, but 8KB strided-destination descriptors
    # (~10% slower) when the source is a column slice of one big tensor
    ot_s = [
        nc.alloc_sbuf_tensor(f"ot{g}", [P, len(ch) * COLS], mybir.dt.float32)
        for g, ch in enumerate(GROUPS)
    ]
    pools = [
        nc.alloc_psum_tensor("pa", [P, 4 * COLS], mybir.dt.float32),
        nc.alloc_psum_tensor("pb", [P, 4 * COLS], mybir.dt.float32),
    ]

    s_in = nc.alloc_semaphore("s_in")  # input DMA completions (+16 each)
    s_pe = nc.alloc_semaphore("s_pe")  # matmul retirements (+1 each)
    s_vc = nc.alloc_semaphore("s_vc")  # vector copy retirements
    s_sc = nc.alloc_semaphore("s_sc")  # scalar copy retirements
    s_do = nc.alloc_semaphore("s_do")  # output DMA completions (+16 each)
    sems = [s_in, s_pe, s_vc, s_sc, s_do]

    nc.sync.dma_start(hd_s[:], hd[:]).then_inc(s_in, 16)
    nc.sync.dma_start(ltt_s[:], lt_t[:]).then_inc(s_in, 16)
    rh_s = hd_s[:, :COLS]

    order = RAMP_CHUNKS + [c for c in range(NB) if c not in RAMP_CHUNKS]
    pos_of = {c: i for i, c in enumerate(order)}

    def lhsT(c):
        i = pos_of[c]
        if i < HEAD_BLOCKS:
            return hd_s[:, COLS + i * P : COLS + (i + 1) * P]
        i -= HEAD_BLOCKS
        return ltt_s[:, i * P : (i + 1) * P]

    n_groups = len(GROUPS)
    copy_sem = lambda g: s_vc if g % 2 == 0 else s_sc
    copy_val = [0] * n_groups
    cnt = {0: 0, 1: 0}
    for g in range(n_groups):
        cnt[g % 2] += 1
        copy_val[g] = cnt[g % 2]

    # per-pool PSUM bank rotation + last-user tracking for WAR waits (every
    # user of a pool has the same parity, so each WAR is a single-sem wait)
    bank_last = [[None] * 4, [None] * 4]
    rot = [0, 0]
    cum_mm = 0
    n_dma = 0
    used_ltt = False
    for g, chunks in enumerate(GROUPS):
        nbl = len(chunks)
        pool = g % 2
        if nbl == 1:
            b0b = rot[pool] % 4
        else:
            b0b = (rot[pool] % 2) * 2
        rot[pool] += 1
        pt = pools[pool][:, b0b * COLS : (b0b + nbl) * COLS]
        # PE gating (waits fuse into the next PE instruction; at most one
        # group carries two waits, costing a single event-sem instruction)
        if g == 0:
            nc.tensor.wait_ge(s_in, 16)  # rh + head lhsT resident
        if not used_ltt and any(pos_of[c] >= HEAD_BLOCKS for c in chunks):
            nc.tensor.wait_ge(s_in, 32)  # lhsT tail resident
            used_ltt = True
        war = [
            bank_last[pool][b]
            for b in range(b0b, b0b + nbl)
            if bank_last[pool][b] is not None
        ]
        if war:
            w = max(war)
            nc.tensor.wait_ge(copy_sem(w), copy_val[w])
        for b in range(b0b, b0b + nbl):
            bank_last[pool][b] = g
        rows = LAST_ROWS if chunks == [NB - 1] else P
        for i, c in enumerate(chunks):
            nc.tensor.matmul(
                pt[:, i * COLS : (i + 1) * COLS],
                lhsT(c),
                rh_s,
                start=True,
                stop=True,
            ).then_inc(s_pe, 1)
        cum_mm += nbl
        ncols = nbl * COLS
        ot = ot_s[g]
        eng = nc.vector if g % 2 == 0 else nc.scalar
        eng.wait_ge(s_pe, cum_mm)
        if g % 2 == 0:
            ci = nc.vector.tensor_copy(ot[:rows, :], pt[:rows, :])
        else:
            ci = nc.scalar.copy(ot[:rows, :], pt[:rows, :])
        ci.then_inc(copy_sem(g), 1)
        # output DMA; multi-chunk groups rearrange so the HBM destination
        # stays one contiguous run (line-rate descriptors, v1-measured)
        nc.sync.wait_ge(copy_sem(g), copy_val[g])
        c0 = chunks[0]
        if nbl == 1:
            dst = out[c0 * P : c0 * P + rows, :]
            src = ot[:rows, :]
        else:
            dst = out[c0 * P : (c0 + nbl) * P, :].rearrange(
                "(j p) q -> p j q", p=P
            )
            src = ot[:, :ncols].rearrange("p (j q) -> p j q", q=COLS)
        nc.sync.dma_start(dst, src).then_inc(s_do, 16)
        n_dma += 1

    # Epilogue: kernel completion = all output bytes landed.  gpsimd then
    # re-zeroes our semaphores so a re-execution of this NEFF starts clean
    # (the framework contract is sems == 0 at kernel entry).
    nc.sync.wait_ge(s_do, 16 * n_dma)
    nc.gpsimd.wait_ge(s_do, 16 * n_dma)
    nums = sorted(s.num for s in sems)
    if nums == list(range(nums[0], nums[0] + len(nums))):
        nc.gpsimd.sem_clear(range(nums[0], nums[-1] + 1))
    else:
        for s in sems:
            nc.gpsimd.sem_clear(s)
    nc.finalize()  # runs Bacc.compile(): reg alloc + sync-wait legalization
    return nc


def _split_bf16(x: np.ndarray, n: int):
    """Split x into n bf16 parts summing (nearly) exactly to x."""
    parts = []
    rem = np.asarray(x, dtype=np.float64).copy()
    for _ in range(n):
        p = rem.astype(_bf16)
        parts.append(p)
        rem = rem - p.astype(np.float64)
    return parts


def _host_tables(pos0: np.ndarray, vel0: np.ndarray):
    """Build per-core input tables (float64 math, cast at the end)."""
    ax, ay = float(pos0[0]), float(pos0[1])
    bx_c = DT * float(vel0[0])  # B_x (C_x = 0)
    by_c = DT * float(vel0[1]) - C_Y  # B_y

    # fixed rhs column patterns (jb < 256 within every 512-col chunk)
    ce = np.arange(COLS)
    j = (ce >> 1).astype(np.float64)
    odd = (ce & 1).astype(np.float64)
    even = 1.0 - odd
    jodd = (j * odd).astype(_bf16)  # exact: j < 256
    resid = np.where(ce & 1 == 1, C_Y * j * j, bx_c * j)
    resid_hi, resid_lo = _split_bf16(resid, 2)
    rh_np = np.stack(
        [
            jodd,
            jodd,
            resid_hi,
            resid_lo,
            odd.astype(_bf16),
            odd.astype(_bf16),
            odd.astype(_bf16),
            even.astype(_bf16),
            even.astype(_bf16),
            even.astype(_bf16),
        ]
    )  # [K, COLS]

    in_maps = []
    c_idx = np.arange(NB, dtype=np.float64)[:, None]  # [NB, 1]
    p_idx = np.arange(P, dtype=np.float64)[None, :]  # [1, P]
    for k in range(N_CORES):
        q = k * (CE // 2) + c_idx * (CHUNK // 2) + p_idx * (COLS // 2)  # [NB, P]
        s1_hi, s1_lo = _split_bf16(by_c + 2.0 * C_Y * q, 2)
        ones = np.ones_like(s1_hi)
        by3 = _split_bf16(ay + by_c * q + C_Y * q * q, 3)
        bx3 = _split_bf16(ax + bx_c * q, 3)
        rows = [s1_hi, s1_lo, ones, ones] + by3 + bx3
        lt_np = np.stack([r.reshape(-1) for r in rows])  # [K, NB*P], chunk-major
        order = RAMP_CHUNKS + [c for c in range(NB) if c not in RAMP_CHUNKS]
        lt_np = np.concatenate(
            [lt_np[:, c * P : (c + 1) * P] for c in order], axis=1
        )
        in_maps.append(
            {
                "hd": np.ascontiguousarray(
                    np.concatenate([rh_np, lt_np[:, : HEAD_BLOCKS * P]], axis=1)
                ),
                "lt_t": np.ascontiguousarray(lt_np[:, HEAD_BLOCKS * P :]),
            }
        )
    return in_maps


def kernel(ball_mass, ball_initial_position, ball_initial_velocity) -> np.ndarray:
    global LAST_RESULTS
    pos0 = np.asarray(ball_initial_position, dtype=np.float32)
    vel0 = np.asarray(ball_initial_velocity, dtype=np.float32)

    _ensure_axon_hooks_stub()
    nc = _build_program()
    in_maps = _host_tables(pos0, vel0)
    res = run_bass_kernel_spmd(nc, in_maps, core_ids=list(range(N_CORES)))
    LAST_RESULTS = res

    parts = [
        np.asarray(r["out"], dtype=np.float32).reshape(-1)[:CE] for r in res.results
    ]
    return np.concatenate(parts).reshape(N_PAIRS, 2)


if __name__ == "__main__":
    import os

    pos0 = (
        np.load("/tmp/pos0.npy")
        if os.path.exists("/tmp/pos0.npy")
        else np.array([-1.866805, -0.25733662], np.float32)
    )
    vel0 = (
        np.load("/tmp/vel0.npy")
        if os.path.exists("/tmp/vel0.npy")
        else np.array([-0.847358, -1.5444987], np.float32)
    )
    outv = kernel(np.ones(()), pos0, vel0)
    i = np.arange(N_PAIRS, dtype=np.float64)[:, None]
    closed = (
        pos0.astype(np.float64)
        + i * DT * vel0.astype(np.float64)
        + np.array([0.0, GDT_Y * DT]) * i * (i - 1) / 2.0
    )
    err = np.abs(outv - closed)
    denom = np.maximum(np.abs(closed), 1e-12)
    print("closed-form maxabs-ratio rel err:", err.max() / np.abs(closed).max())
    print("closed-form max elementwise rel err:", (err / denom).max())
